# revision 1
# baseline (speedup 1.0000x reference)
"""Trainium2 Bass kernel for nn_ARIG_Fusion (dual sigmoid gating + proj + BatchNorm + LIF).

Strategy (8 NeuronCores, SPMD):
  - Shard batch B=32 into 8 shards of 4. Each core handles rows (t, b_loc, n)
    = 4*4*1024 = 16384 rows of C=256 channels.
  - All tensors live on-chip in TRANSPOSED layout [channel, row]; the host
    pre-transposes inputs and post-transposes outputs.
  - Inputs stay f32 in HBM/SBUF but are DECLARED f32r, so the gate GEMMs
    consume them directly (no rounding copies; f32r = same bits, 1 cyc/row
    on the PE vs 4 for fp32). The projection GEMM also runs f32r: the fused
    tensor's final add writes an f32r-typed tile (DVE converts on write).
  - b_proj is dropped on device: BatchNorm subtracts the per-channel mean,
    so a per-channel bias before BN cancels exactly.
  - outb is stored as int16 (Act Copy with scale=S): absolute quantization
    error (1/2S ~ 4e-5) beats fp16's relative error ~8x, halves SBUF, and
    BN's affine invariance absorbs the scale (only EPS must be rescaled).
  - BatchNorm stats: per-tile bn_stats on the int16 outb -> bn_aggr, then a
    [128,4] AllReduce across 8 cores combines (mean, E[x^2]) in quant units.
  - LIF (T=4, rescaled state U_t = v_t/(1-tau)^t): per t, Act dequantizes +
    applies the BN affine (ay = asc_t*outb + ash_t, per-channel scale/bias
    APs), DVE does U = ay + W (f32 tensor_tensor), s' = (U < th_t) with u8
    output (tensor_scalar, 2x mode; u8 quarters the spike DMA), and
    W = U * s' (the INVERTED spike gives the hard reset directly; the host
    emits 1 - s'). The W multiply can be split DVE/gpsimd via W_POOL_COLS.
"""

import math

import numpy as np

T, B, N, C = 4, 32, 1024, 256
NCORES = 8
BL = B // NCORES          # 4 batches per core
R = T * BL * N            # 16384 rows per core
RT = BL * N               # 4096 rows per t-slice
F = 512                   # gating tile columns
NT = R // F               # 32 gating tiles
CC = 1024                 # LIF column chunk (nq = RT // CC)
EPS = 1e-5
V_TH = 1.0
QS = 12288.0              # outb int16 quantization scale (range ~±2.67)

_program_cache = {}

# tuning knobs (read at trace time)
GIN_BUFS = 3
GATE_BUFS = 2
V_BUFS = 2
PG_BUFS = 1
PO_BUFS = 2
LIF_BUFS = 2
AY_BUFS = 2
V2_ENGINE = "dve"        # gpsimd | dve
COPY_SPLIT = "act"       # act | act+dve
ADD_SPLIT = "dve"        # dve | dve+pool
S_BUFS = 2
W_POOL_COLS = 0          # LIF W-mult cols sent to gpsimd (of CC)
PROJ_MODE = "f32r"       # f32r | split | f32
STAGGER = 1              # software-pipeline proj/copy/bn by 1-2 tiles
MERGE_IN_DMA = 1         # single combined a+l DMA per tile
U_VIA_DMA = 0            # LIF U-add via DMA accumulate
SIG_BIAS_MM = 0          # gate bias via k=1 ones-matmul, merged sigmoids
PROJ_PRODUCTS = 0        # project v1,v2 separately (PSUM accumulates, no add)
PHASES = ("gate", "fin", "lif")


def _build_program(tau_inv: float, reps: int = 1, single_core: bool = False):
    do_fin = "fin" in PHASES
    do_lif = "lif" in PHASES
    cc = CC
    nq = RT // cc
    import concourse.bacc as bacc
    import concourse.bass as bass
    import concourse.tile as tile
    from concourse import mybir

    f32 = mybir.dt.float32
    f32r = mybir.dt.float32r
    i16 = mybir.dt.int16
    u8 = mybir.dt.uint8
    Alu = mybir.AluOpType
    Act = mybir.ActivationFunctionType

    # LIF rescaling: U_t = v_t / (1-tau)^t;  U_t = W_{t-1} + alpha_t * y_t
    one_m = 1.0 - tau_inv
    alphas = [tau_inv / (one_m ** t) for t in range(T)]
    ths = [V_TH / (one_m ** t) for t in range(T)]

    nc = bacc.Bacc("TRN2", target_bir_lowering=False, debug=False,
                   num_devices=1 if single_core else NCORES)

    proj_dt = f32 if PROJ_MODE == "f32" else f32r
    if MERGE_IN_DMA:
        xt_d = nc.dram_tensor("xt", [2, 2, 128, R], f32r,
                              kind="ExternalInput")
    else:
        at_d = nc.dram_tensor("at", [2, 128, R], f32r, kind="ExternalInput")
        lt_d = nc.dram_tensor("lt", [2, 128, R], f32r, kind="ExternalInput")
    w1_d = nc.dram_tensor("w1", [128, 2, 2, 128], f32r, kind="ExternalInput")
    w2_d = nc.dram_tensor("w2", [128, 2, 2, 128], f32r, kind="ExternalInput")
    nw3 = 2 if PROJ_MODE == "split" else 1
    w3_d = nc.dram_tensor("w3", [128, nw3, 2, 2, 128], proj_dt,
                          kind="ExternalInput")
    pp_d = nc.dram_tensor("pp", [128, 10], f32, kind="ExternalInput")
    if SIG_BIAS_MM:
        bm_d = nc.dram_tensor("bm", [1, 2, 2, 128], f32r,
                              kind="ExternalInput")
        on_d = nc.dram_tensor("on", [1, F], f32r, kind="ExternalInput")
    sp_d = nc.dram_tensor("sp", [2, 128, R], u8, kind="ExternalOutput")

    with tile.TileContext(nc) as tc:
      for _rep in range(reps):
        with tc.tile_pool(name="singles", bufs=1) as singles:
            w1s = singles.tile([128, 2, 2, 128], f32r)
            w2s = singles.tile([128, 2, 2, 128], f32r)
            w3s = singles.tile([128, nw3, 2, 2, 128], proj_dt)
            pps = singles.tile([128, 10], f32)
            outb = singles.tile([128, 2, R], i16)
            stb = singles.tile([128, 2, NT, 6], f32)
            nc.sync.dma_start(w1s[:], w1_d[:, :, :, :])
            nc.sync.dma_start(w2s[:], w2_d[:, :, :, :])
            nc.sync.dma_start(w3s[:], w3_d[:, :, :, :, :])
            nc.sync.dma_start(pps[:], pp_d[:, :])
            if SIG_BIAS_MM:
                bms = singles.tile([1, 2, 2, 128], f32r)
                ons = singles.tile([1, F], f32r)
                nc.sync.dma_start(bms[:], bm_d[:, :, :, :])
                nc.sync.dma_start(ons[:], on_d[:, :])

            # ---------------- gating + projection + stats ----------------
            def pjc(dram_ap):
                # [j, p, c] DRAM slice -> [p, j, c] access pattern
                return bass.AP(tensor=dram_ap.tensor, offset=dram_ap.offset,
                               ap=[dram_ap.ap[1], dram_ap.ap[0], dram_ap.ap[2]])

            with (
                tc.tile_pool(name="gin", bufs=GIN_BUFS) as gin,
                tc.tile_pool(name="gate", bufs=GATE_BUFS) as gatep,
                tc.tile_pool(name="vp", bufs=V_BUFS) as vp,
                tc.tile_pool(name="pg", bufs=PG_BUFS, space="PSUM") as pg,
                tc.tile_pool(name="po", bufs=PO_BUFS, space="PSUM") as po,
            ):
                # Software-pipelined: proj matmuls, the outb copy, and
                # bn_stats for tile k are emitted in LATER iterations so no
                # engine head-of-line blocks on a same-tile dependency.
                def emit_proj(k, vrk, v2k=None):
                    p3 = po.tile([128, 2, F], f32, tag="o", name="po")
                    for j in (0, 1):
                        nc.tensor.matmul(p3[:, j, :], w3s[:, 0, 0, j, :],
                                         vrk[:, 0, :], start=True, stop=False)
                        last = (PROJ_MODE != "split") and v2k is None
                        nc.tensor.matmul(p3[:, j, :], w3s[:, 0, 1, j, :],
                                         vrk[:, 1, :], start=False, stop=last)
                        if v2k is not None:
                            nc.tensor.matmul(p3[:, j, :], w3s[:, 0, 0, j, :],
                                             v2k[:, 0, :], start=False,
                                             stop=False)
                            nc.tensor.matmul(p3[:, j, :], w3s[:, 0, 1, j, :],
                                             v2k[:, 1, :], start=False,
                                             stop=(PROJ_MODE != "split"))
                        if PROJ_MODE == "split":
                            nc.tensor.matmul(p3[:, j, :], w3s[:, 1, 0, j, :],
                                             vrk[:, 0, :], start=False,
                                             stop=False)
                            nc.tensor.matmul(p3[:, j, :], w3s[:, 1, 1, j, :],
                                             vrk[:, 1, :], start=False,
                                             stop=v2k is None)
                            if v2k is not None:
                                nc.tensor.matmul(p3[:, j, :],
                                                 w3s[:, 1, 0, j, :],
                                                 v2k[:, 0, :], start=False,
                                                 stop=False)
                                nc.tensor.matmul(p3[:, j, :],
                                                 w3s[:, 1, 1, j, :],
                                                 v2k[:, 1, :], start=False,
                                                 stop=True)
                    return p3

                def emit_copy(k, p3k):
                    # quantize to int16 (scale QS); b_proj dropped (BN cancels)
                    slk = slice(k * F, (k + 1) * F)
                    if COPY_SPLIT == "act":
                        nc.scalar.activation(outb[:, :, slk], p3k[:], Act.Copy,
                                             scale=QS)
                    else:
                        nc.scalar.activation(outb[:, 0, slk], p3k[:, 0, :],
                                             Act.Copy, scale=QS)
                        eng = (nc.gpsimd if COPY_SPLIT == "act+pool"
                               else nc.vector)
                        eng.tensor_scalar(outb[:, 1, slk], p3k[:, 1, :],
                                          QS, None, Alu.mult)

                def emit_bn(k):
                    slk = slice(k * F, (k + 1) * F)
                    for j in (0, 1):
                        nc.vector.bn_stats(stb[:, j, k, :], outb[:, j, slk])

                vr_q = [None, None]   # vr tiles awaiting proj
                p3_q = [None, None]   # p3 tiles awaiting copy
                for i in range(NT):
                    sl = slice(i * F, (i + 1) * F)
                    if MERGE_IN_DMA:
                        xt = gin.tile([128, 2, 2, F], f32r, tag="xt",
                                      name="xt")
                        dap = xt_d[:, :, :, sl]
                        nc.sync.dma_start(
                            xt[:], bass.AP(tensor=dap.tensor,
                                           offset=dap.offset,
                                           ap=[dap.ap[2], dap.ap[0],
                                               dap.ap[1], dap.ap[3]]))
                        a2 = xt[:, 0]
                        l2 = xt[:, 1]
                    else:
                        a2t = gin.tile([128, 2, F], f32r, tag="a2", name="a2")
                        l2t = gin.tile([128, 2, F], f32r, tag="l2", name="l2")
                        nc.sync.dma_start(a2t[:], pjc(at_d[:, :, sl]))
                        nc.sync.dma_start(l2t[:], pjc(lt_d[:, :, sl]))
                        a2 = a2t[:]
                        l2 = l2t[:]
                    g1 = pg.tile([128, 2, F], f32, tag="g1", name="g1")
                    g2 = pg.tile([128, 2, F], f32, tag="g2", name="g2")
                    for j in (0, 1):
                        if SIG_BIAS_MM:
                            nc.tensor.matmul(g1[:, j, :], bms[:, 0, j, :],
                                             ons[:, :], start=True, stop=False)
                        nc.tensor.matmul(g1[:, j, :], w1s[:, 0, j, :],
                                         a2[:, 0, :], start=not SIG_BIAS_MM,
                                         stop=False)
                        nc.tensor.matmul(g1[:, j, :], w1s[:, 1, j, :],
                                         a2[:, 1, :], start=False, stop=True)
                    if STAGGER and i > 0:
                        p3_q[1] = (emit_proj(i - 1, *vr_q[1]) if PROJ_PRODUCTS
                                   else emit_proj(i - 1, vr_q[1]))
                    for j in (0, 1):
                        if SIG_BIAS_MM:
                            nc.tensor.matmul(g2[:, j, :], bms[:, 1, j, :],
                                             ons[:, :], start=True, stop=False)
                        nc.tensor.matmul(g2[:, j, :], w2s[:, 0, j, :],
                                         l2[:, 0, :], start=not SIG_BIAS_MM,
                                         stop=False)
                        nc.tensor.matmul(g2[:, j, :], w2s[:, 1, j, :],
                                         l2[:, 1, :], start=False, stop=True)
                    gL = gatep.tile([128, 2, F], f32, tag="gL", name="gL")
                    gA = gatep.tile([128, 2, F], f32, tag="gA", name="gA")
                    if SIG_BIAS_MM:
                        nc.scalar.activation(gL[:], g1[:], Act.Sigmoid)
                        nc.scalar.activation(gA[:], g2[:], Act.Sigmoid)
                    else:
                        for j in (0, 1):
                            nc.scalar.activation(gL[:, j, :], g1[:, j, :],
                                                 Act.Sigmoid,
                                                 bias=pps[:, 0 + j:1 + j])
                            nc.scalar.activation(gA[:, j, :], g2[:, j, :],
                                                 Act.Sigmoid,
                                                 bias=pps[:, 2 + j:3 + j])
                    if STAGGER and i > 0:
                        emit_copy(i - 1, p3_q[1])
                    if STAGGER and i > 1:
                        emit_bn(i - 2)
                    if PROJ_PRODUCTS:
                        v1 = vp.tile([128, 2, F], proj_dt, tag="v1",
                                     name="v1")
                        v2 = vp.tile([128, 2, F], proj_dt, tag="v2",
                                     name="v2")
                        nc.vector.tensor_mul(v1[:], a2.bitcast(f32), gA[:])
                        if V2_ENGINE == "gpsimd":
                            nc.gpsimd.tensor_mul(v2[:], l2.bitcast(f32),
                                                 gL[:])
                        else:
                            nc.vector.tensor_mul(v2[:], l2.bitcast(f32),
                                                 gL[:])
                        vr_q[1] = (v1, v2)
                    else:
                        v1 = vp.tile([128, 2, F], f32, tag="v1", name="v1")
                        v2 = vp.tile([128, 2, F], f32, tag="v2", name="v2")
                        vr = vp.tile([128, 2, F], proj_dt, tag="vr",
                                     name="vr")
                        nc.vector.tensor_mul(v1[:], a2.bitcast(f32), gA[:])
                        if V2_ENGINE == "gpsimd":
                            nc.gpsimd.tensor_mul(v2[:], l2.bitcast(f32),
                                                 gL[:])
                        else:
                            nc.vector.tensor_mul(v2[:], l2.bitcast(f32),
                                                 gL[:])
                        if ADD_SPLIT == "dve+pool":
                            nc.vector.tensor_add(vr[:, 0, :], v1[:, 0, :],
                                                 v2[:, 0, :])
                            nc.gpsimd.tensor_add(vr[:, 1, :], v1[:, 1, :],
                                                 v2[:, 1, :])
                        else:
                            nc.vector.tensor_add(vr[:], v1[:], v2[:])
                        vr_q[1] = vr
                    if not STAGGER:
                        p3 = (emit_proj(i, *vr_q[1]) if PROJ_PRODUCTS
                              else emit_proj(i, vr_q[1]))
                        emit_copy(i, p3)
                        emit_bn(i)
                if STAGGER:
                    # drain: last tile's proj/copy/bn
                    p3_last = (emit_proj(NT - 1, *vr_q[1]) if PROJ_PRODUCTS
                               else emit_proj(NT - 1, vr_q[1]))
                    emit_copy(NT - 1, p3_last)
                    emit_bn(NT - 2)
                    emit_bn(NT - 1)
            # ---------------- stats finalize + all-reduce ----------------
            if not do_fin:
                continue
            with (
                tc.tile_pool(name="fin", bufs=1) as fin,
                tc.tile_pool(name="dramp", bufs=1, space="DRAM") as dramp,
            ):
                mv = fin.tile([128, 2, 2], f32)
                ccs = fin.tile([128, 4], f32)
                for j in (0, 1):
                    nc.vector.bn_aggr(mv[:, j, :], stb[:, j, :, :])
                    nc.vector.tensor_copy(ccs[:, 2 * j:2 * j + 1], mv[:, j, 0:1])
                    nc.vector.tensor_mul(ccs[:, 2 * j + 1:2 * j + 2],
                                         mv[:, j, 0:1], mv[:, j, 0:1])
                    nc.vector.tensor_add(ccs[:, 2 * j + 1:2 * j + 2],
                                         ccs[:, 2 * j + 1:2 * j + 2],
                                         mv[:, j, 1:2])
                if single_core:
                    cg = fin.tile([128, 4], f32)
                    nc.vector.tensor_scalar(cg[:], ccs[:], float(NCORES),
                                            None, Alu.mult)
                else:
                    cc_in = dramp.tile([128, 4], f32)
                    cc_out = dramp.tile([128, 4], f32)
                    nc.gpsimd.dma_start(cc_in[:], ccs[:])
                    nc.gpsimd.collective_compute(
                        "AllReduce", Alu.add,
                        replica_groups=[list(range(NCORES))],
                        ins=[cc_in.opt()], outs=[cc_out.opt()],
                    )
                    cg = fin.tile([128, 4], f32)
                    nc.gpsimd.dma_start(cg[:], cc_out[:])

                mean = fin.tile([128, 2], f32)
                varp = fin.tile([128, 2], f32)
                sc = fin.tile([128, 2], f32)
                sh = fin.tile([128, 2], f32)
                t1 = fin.tile([128, 2], f32)
                t2 = fin.tile([128, 2], f32)
                r0 = fin.tile([128, 2], f32)
                for j in (0, 1):
                    jm = slice(j, j + 1)
                    nc.vector.tensor_scalar(mean[:, jm], cg[:, 2 * j:2 * j + 1],
                                            1.0 / NCORES, None, Alu.mult)
                    # varp = E[x^2] - mean^2 + eps  (in quant units: eps*QS^2)
                    nc.vector.tensor_scalar(varp[:, jm],
                                            cg[:, 2 * j + 1:2 * j + 2],
                                            1.0 / NCORES, None, Alu.mult)
                    nc.vector.tensor_mul(t1[:, jm], mean[:, jm], mean[:, jm])
                    nc.vector.tensor_sub(varp[:, jm], varp[:, jm], t1[:, jm])
                    nc.vector.tensor_scalar(varp[:, jm], varp[:, jm],
                                            EPS * QS * QS, None, Alu.add)
                # r0 = 1/sqrt(varp), via ACT sqrt + reciprocal + 2 Newton steps
                nc.scalar.activation(r0[:], varp[:], Act.Sqrt)
                nc.vector.reciprocal(r0[:], r0[:])
                for _ in range(2):
                    nc.vector.tensor_mul(t1[:], r0[:], r0[:])
                    nc.vector.tensor_mul(t2[:], t1[:], varp[:])
                    nc.vector.tensor_scalar(t2[:], t2[:], -0.5, 1.5,
                                            Alu.mult, Alu.add)
                    nc.vector.tensor_mul(r0[:], r0[:], t2[:])
                # sc = gamma * r0 (y = sc*outb_q + sh with outb_q in quant
                # units; the QS scale folds into r0 automatically)
                for j in (0, 1):
                    jm = slice(j, j + 1)
                    nc.vector.tensor_mul(sc[:, jm], pps[:, 6 + j:7 + j],
                                         r0[:, jm])
                    nc.vector.tensor_mul(t1[:, jm], mean[:, jm], sc[:, jm])
                    nc.vector.tensor_sub(sh[:, jm], pps[:, 8 + j:9 + j],
                                         t1[:, jm])

                # per-t pre-scaled BN affine: ay_t = alpha_t*(sc*outb+sh)
                asc = fin.tile([128, T, 2], f32)
                ash = fin.tile([128, T, 2], f32)
                for t in range(T):
                    for j in (0, 1):
                        nc.vector.tensor_scalar(asc[:, t, j:j + 1], sc[:, j:j + 1],
                                                alphas[t], None, Alu.mult)
                        nc.vector.tensor_scalar(ash[:, t, j:j + 1], sh[:, j:j + 1],
                                                alphas[t], None, Alu.mult)

                # ---------------- LIF scan + spike output ----------------
                # s' = (U < th) is the INVERTED spike: W = U*s' gives the
                # hard reset, and the host emits spikes = 1 - s'.
                if not do_lif:
                    continue
                # t-outer / q-inner so the Pool W-multiply latency of chunk q
                # hides behind the other chunks' DVE work.
                with tc.tile_pool(name="lif", bufs=LIF_BUFS) as lifp:
                    wprev = [None] * nq
                    for t in range(T):
                        for q in range(nq):
                            csl = slice(t * RT + q * cc, t * RT + (q + 1) * cc)
                            ay = lifp.tile([128, 2, cc], f32, tag="ay",
                                           name="ay", bufs=AY_BUFS)
                            for j in (0, 1):
                                nc.scalar.activation(
                                    ay[:, j, :], outb[:, j, csl], Act.Identity,
                                    bias=ash[:, t, j:j + 1],
                                    scale=asc[:, t, j:j + 1])
                            if t == 0:
                                u = ay
                            elif U_VIA_DMA:
                                nc.gpsimd.dma_start(ay[:], wprev[q][:],
                                                    accum_op=Alu.add)
                                u = ay
                            else:
                                u = lifp.tile([128, 2, cc], f32, tag="U",
                                              name="U")
                                nc.vector.tensor_add(u[:], ay[:], wprev[q][:])
                            s = lifp.tile([128, 2, cc], u8, tag="s", name="s",
                                          bufs=S_BUFS)
                            nc.vector.tensor_scalar(
                                s[:], u[:], float(ths[t]), None, Alu.is_lt)
                            nc.sync.dma_start(pjc(sp_d[:, :, csl]), s[:])
                            if t < T - 1:
                                wn = lifp.tile([128, 2, cc], f32,
                                               tag=f"W{q}", name=f"Wn{q}",
                                               bufs=1)
                                if W_POOL_COLS > 0:
                                    c0 = cc - W_POOL_COLS
                                    if c0 > 0:
                                        nc.vector.tensor_mul(
                                            wn[:, :, :c0], u[:, :, :c0],
                                            s[:, :, :c0])
                                    nc.gpsimd.tensor_mul(
                                        wn[:, :, c0:], u[:, :, c0:],
                                        s[:, :, c0:])
                                else:
                                    nc.vector.tensor_mul(wn[:], u[:], s[:])
                                wprev[q] = wn

    nc.compile()
    return nc


def _get_program(tau_inv: float, reps: int = 1, single_core: bool = False):
    key = (round(float(tau_inv), 12), reps, single_core)
    if key not in _program_cache:
        _program_cache[key] = _build_program(float(tau_inv), reps, single_core)
    return _program_cache[key]


def _shard_transpose(x):
    # [T,B,N,C] -> [cores, 2, 128, R] with rows ordered (t, b_loc, n)
    v = x.reshape(T, NCORES, BL, N, C)
    v = np.transpose(v, (1, 4, 0, 2, 3))
    return np.ascontiguousarray(v).reshape(NCORES, 2, 128, R)


def _prep_w(w):
    # lhsT chunks [p, k, j, q]: W.T viewed as [k,128p][j,128q]
    wt = np.ascontiguousarray(w.T).reshape(2, 128, 2, 128)
    return np.ascontiguousarray(wt.transpose(1, 0, 2, 3))


def _two(vec):
    return np.ascontiguousarray(vec.reshape(2, 128).T)


def _make_in_maps(inputs):
    x_attn = np.asarray(inputs["x_attn"], dtype=np.float32)
    x_lsm = np.asarray(inputs["x_lsm"], dtype=np.float32)
    at = _shard_transpose(x_attn)
    lt = _shard_transpose(x_lsm)
    xt = np.stack([at, lt], axis=1)  # [cores, 2, 2, 128, R]
    w1 = _prep_w(np.asarray(inputs["W_att"], dtype=np.float32))
    w2 = _prep_w(np.asarray(inputs["W_lsm"], dtype=np.float32))
    w3full = _prep_w(np.asarray(inputs["W_proj"], dtype=np.float32))
    if PROJ_MODE == "split":
        # hi/lo weight split: hi = bf16-rounded, lo = residual
        hi = w3full.astype(np.float32)
        hi = (hi.view(np.uint32) & np.uint32(0xFFFF0000)).view(np.float32)
        w3 = np.stack([hi, w3full - hi], axis=1)
    else:
        w3 = w3full[:, None]
    pp = np.concatenate(
        [_two(np.asarray(inputs["b_att"], dtype=np.float32)),
         _two(np.asarray(inputs["b_lsm"], dtype=np.float32)),
         _two(np.asarray(inputs["b_proj"], dtype=np.float32)),
         _two(np.asarray(inputs["gamma"], dtype=np.float32)),
         _two(np.asarray(inputs["beta"], dtype=np.float32))],
        axis=1)
    base = {"w1": w1, "w2": w2, "w3": w3, "pp": pp}
    if SIG_BIAS_MM:
        bm = np.stack([
            np.asarray(inputs["b_att"], dtype=np.float32).reshape(2, 128),
            np.asarray(inputs["b_lsm"], dtype=np.float32).reshape(2, 128),
        ])[None]  # [1, 2, 2, 128]
        base["bm"] = np.ascontiguousarray(bm)
        base["on"] = np.ones((1, F), dtype=np.float32)
    if MERGE_IN_DMA:
        return [dict(base, xt=xt[s]) for s in range(NCORES)]
    return [dict(base, at=at[s], lt=lt[s]) for s in range(NCORES)]


def kernel(**inputs):
    from concourse.bass_utils import run_bass_kernel_spmd

    lif_w = float(np.asarray(inputs["lif_w"], dtype=np.float32))
    tau_inv = float(np.float32(1.0 / (1.0 + math.exp(-lif_w))))
    nc = _get_program(tau_inv)
    in_maps = _make_in_maps(inputs)
    res = run_bass_kernel_spmd(nc, in_maps, core_ids=list(range(NCORES)))
    kernel.last_results = res

    S = np.stack([r["sp"] for r in res.results]).reshape(
        NCORES, 2, 128, T, BL, N)
    out = np.transpose(S, (3, 0, 4, 5, 1, 2))
    # sp holds the inverted spike s' = (U < th); emit 1 - s'
    return (1 - np.ascontiguousarray(out).reshape(T, B, N, C)).astype(
        np.float32)

